# revision 25
# baseline (speedup 1.0000x reference)
"""Trainium2 Bass kernel for CohereAttention (T=2048, H=4096, NH=32, NKV=8, HD=128).

Sharding: tensor-parallel across heads on 8 cores (SGLang-style).
  - core c owns q-heads [4c, 4c+4) and kv-head c (GQA rep=4 maps exactly).
  - w_qkv column-sharded per core -> [4096, 768] (4q|1k|1v head blocks).
  - attention output (bf16, d-major [512, 2048]) AllGather'd across cores.
  - w_o column-sharded -> each core computes a [2048, 512] column shard of the
    output (stored transposed [512, 2048]); host concatenates.

Device pipeline per core:
  P1  qkv = hiddenT.T @ w_qkv_shard (bf16 matmul, fp32 psum), per-head
      layernorm + interleaved (GPT-J) RoPE fused on DVE/ACT, V cast to bf16.
  P2  PE transposes q/k head tiles [t,d] -> [d,t] for the scores matmuls.
  P3  per head: scoresT[k,q] = K_d^T Q_d; probsT = exp(scale*s) (causal-masked);
      attnT[d,q] += V_k^T probsT; sums[q] += ones^T probsT (PSUM accumulate);
      attnT_norm = attnT * recip(sums) -> bf16 -> DRAM AllGather input.
  P4  AllGather (bf16, 2.1MB/core) -> [4096, 2048].
  P5  o_projT[hcol, t] += w_o[:, hcol]^T attnT_full, fp32 out.
"""

import numpy as np
import ml_dtypes

T = 2048
H = 4096
NH = 32
NKV = 8
HD = 128
N_CORES = 8
QH = NH // N_CORES          # q heads per core = 4
LNH = QH + 1                # layernormed heads per core (4 q + 1 k)
EPS = 1e-5
THETA = 10000.0
SCALE = HD ** -0.5
TT = T // 128               # 16 token tiles
KO = H // 128               # 32 contraction chunks
QC = T // 512               # 4 query chunks of 512
BF16 = ml_dtypes.bfloat16

_CACHE = {}


def _build():
    import concourse.bass as bass
    import concourse.mybir as mybir
    import concourse.tile as tile
    from concourse import bacc
    from contextlib import ExitStack

    dt = mybir.dt
    f32 = dt.float32
    bf16 = dt.bfloat16
    AX = mybir.AxisListType
    OP = mybir.AluOpType
    ACT = mybir.ActivationFunctionType

    nc = bacc.Bacc("TRN2", target_bir_lowering=False, debug=False,
                   num_devices=N_CORES)

    # ---- I/O ----
    hT = nc.dram_tensor("hT", [TT, 128, KO, 128], bf16, kind="ExternalInput")
    wqkv = nc.dram_tensor("wqkv", [H, 768], bf16, kind="ExternalInput")
    wo = nc.dram_tensor("wo", [H, 512], bf16, kind="ExternalInput")
    cosd = nc.dram_tensor("cosd", [128, TT, 64], f32, kind="ExternalInput")
    sind = nc.dram_tensor("sind", [128, TT, 64], f32, kind="ExternalInput")
    lnw = nc.dram_tensor("lnw", [128, LNH, 128], f32, kind="ExternalInput")
    triu = nc.dram_tensor("triu", [128, 128], bf16, kind="ExternalInput")
    ident = nc.dram_tensor("ident", [128, 128], bf16, kind="ExternalInput")
    onesd = nc.dram_tensor("onesd", [128, 128], bf16, kind="ExternalInput")
    outT = nc.dram_tensor("outT", [512, T], f32, kind="ExternalOutput")

    with tile.TileContext(nc) as tc, ExitStack() as ctx:
        const = ctx.enter_context(tc.tile_pool(name="const", bufs=1))
        dram = ctx.enter_context(tc.tile_pool(name="dram", bufs=1, space="DRAM"))

        cos_sb = const.tile([128, TT, 64], f32)
        nc.sync.dma_start(cos_sb[:], cosd.ap())
        sin_sb = const.tile([128, TT, 64], f32)
        nc.sync.dma_start(sin_sb[:], sind.ap())
        lnw_sb = const.tile([128, LNH, 128], f32)
        nc.sync.dma_start(lnw_sb[:], lnw.ap())
        triu_sb = const.tile([128, 128], bf16)
        nc.sync.dma_start(triu_sb[:], triu.ap())
        ident_sb = const.tile([128, 128], bf16)
        nc.sync.dma_start(ident_sb[:], ident.ap())
        ones_sb = const.tile([128, 128], bf16)
        nc.sync.dma_start(ones_sb[:], onesd.ap())
        eps_sb = const.tile([128, 1], f32)
        nc.vector.memset(eps_sb[:], EPS)

        ag_in = [dram.tile([QH * 128, 512], bf16, name=f"agi{i}")
                 for i in range(QC)]
        ag_out = [dram.tile([NH * 128, 512], bf16, addr_space="Shared",
                            name=f"ago{i}")
                  for i in range(QC)]

        # ---- P1: qkv projection + LN + RoPE + transposes, with attention
        # blocks interleaved (block qc emitted once token tiles <= 4qc+3 are
        # done, so its AllGather fires early and hides under P1 compute) ----
        with tc.tile_pool(name="htp", bufs=3) as htp, \
             tc.tile_pool(name="qkps", bufs=2, space="PSUM") as qkps, \
             tc.tile_pool(name="p1t", bufs=2) as p1t, \
             tc.tile_pool(name="sps", bufs=2, space="PSUM") as sps, \
             tc.tile_pool(name="pvsm", bufs=1, space="PSUM") as pvsm, \
             tc.tile_pool(name="probs", bufs=3) as probs, \
             tc.tile_pool(name="attn", bufs=2) as attn, \
             tc.tile_pool(name="acts", bufs=1) as acts:

            # persistent activations: d-major Q/K, t-major V (bf16)
            QT = acts.tile([128, QH, TT, 128], bf16)    # [d, h, tt, t]
            KT = acts.tile([128, TT, 128], bf16)        # [d, kt, t]
            Vt = acts.tile([128, TT, 128], bf16)        # [t, kt, d]

            def attn_block(qc):
                for h in range(QH):
                    pvs = pvsm.tile([128, 1024], f32, tag="pvsm")
                    pv = pvs[:, 0:512]
                    sm = pvs[:, 512:1024]
                    nkt = 4 * (qc + 1)
                    for kt in range(nkt):
                        # diagonal band: only q-subtiles j >= m are visible
                        m = max(kt - 4 * qc, 0)
                        lo = m * 128
                        ss = sps.tile([128, 512], f32, tag="ss")
                        nc.tensor.matmul(ss[:, lo:512], KT[:, kt, :],
                                         QT[:, h, 4 * qc + m:4 * qc + 4, :],
                                         start=True, stop=True)
                        pT = probs.tile([128, 4, 128], bf16, tag="pT")
                        pTf = pT.rearrange("p a b -> p (a b)")
                        nc.scalar.activation(pTf[:, lo:512], ss[:, lo:512],
                                             ACT.Exp, scale=SCALE)
                        if kt >= 4 * qc:
                            nc.vector.tensor_tensor(pT[:, m, :], pT[:, m, :],
                                                    triu_sb[:], OP.mult)
                        nc.tensor.matmul(pv[:, lo:512], Vt[:, kt, :],
                                         pTf[:, lo:512],
                                         start=(kt == 0), stop=(kt == nkt - 1))
                        nc.tensor.matmul(sm[:, lo:512], ones_sb[:],
                                         pTf[:, lo:512],
                                         start=(kt == 0), stop=(kt == nkt - 1))
                    recip = attn.tile([128, 512], f32, tag="recip")
                    nc.vector.reciprocal(recip[:], sm[:])
                    at = attn.tile([128, 512], bf16, tag="at")
                    nc.vector.tensor_tensor(at[:], pv[:], recip[:], OP.mult)
                    nc.sync.dma_start(ag_in[qc][h * 128:(h + 1) * 128, :], at[:])

                # AllGather this query block's attention output across cores
                nc.gpsimd.collective_compute(
                    "AllGather", mybir.AluOpType.bypass,
                    replica_groups=[list(range(N_CORES))],
                    ins=[ag_in[qc].opt()], outs=[ag_out[qc].opt()])

            wqkv_r = wqkv.ap().rearrange("(ko p) n -> p ko n", p=128)
            wqkv_sb = htp.tile([128, KO, 768], bf16, tag="wqkv", bufs=1)
            for c in range(8):
                nc.sync.dma_start(wqkv_sb[:, 4 * c:4 * (c + 1), :],
                                  wqkv_r[:, 4 * c:4 * (c + 1), :])
            for tt in range(TT):
                ht_t = htp.tile([128, KO, 128], bf16, tag="ht")
                nc.sync.dma_start(ht_t[:], hT.ap()[tt])
                ps = qkps.tile([128, 768], f32, tag="qk")
                for ko in range(KO):
                    nc.tensor.matmul(ps[:, 0:512], ht_t[:, ko, :],
                                     wqkv_sb[:, ko, 0:512],
                                     start=(ko == 0), stop=(ko == KO - 1))
                    nc.tensor.matmul(ps[:, 512:768], ht_t[:, ko, :],
                                     wqkv_sb[:, ko, 512:768],
                                     start=(ko == 0), stop=(ko == KO - 1))
                qkv_t = p1t.tile([128, 768], f32, tag="qkv")
                nc.scalar.copy(qkv_t[:], ps[:])

                # V: plain bf16 cast into persistent tile
                nc.vector.tensor_copy(Vt[:, tt, :], qkv_t[:, 640:768])

                # layernorm over the 5 q/k heads
                x5 = qkv_t[:, 0:640].rearrange("p (h d) -> p h d", d=128)
                mean = p1t.tile([128, LNH], f32, tag="mean")
                nc.vector.tensor_reduce(mean[:], x5, AX.X, OP.add)
                nc.scalar.mul(mean[:], mean[:], 1.0 / HD)
                xc = p1t.tile([128, LNH, 128], f32, tag="xc")
                nc.vector.tensor_tensor(
                    xc[:], x5, mean[:, :, None].to_broadcast((128, LNH, 128)),
                    OP.subtract)
                sq = p1t.tile([128, LNH, 128], f32, tag="sq")
                nc.vector.tensor_tensor(sq[:], xc[:], xc[:], OP.mult)
                var = p1t.tile([128, LNH], f32, tag="var")
                nc.vector.tensor_reduce(var[:], sq[:], AX.X, OP.add)
                std = p1t.tile([128, LNH], f32, tag="std")
                nc.scalar.activation(std[:], var[:], ACT.Sqrt,
                                     bias=eps_sb[:], scale=1.0 / HD)
                rstd = p1t.tile([128, LNH], f32, tag="rstd")
                nc.vector.reciprocal(rstd[:], std[:])
                nc.vector.tensor_tensor(
                    xc[:], xc[:], rstd[:, :, None].to_broadcast((128, LNH, 128)),
                    OP.mult)
                nc.vector.tensor_tensor(xc[:], xc[:], lnw_sb[:], OP.mult)

                # interleaved RoPE: out[2i] = x1*cos - x2*sin; out[2i+1] = x2*cos + x1*sin
                x1 = xc[:, :, 0:128:2]
                x2 = xc[:, :, 1:128:2]
                cos_b = cos_sb[:, tt:tt + 1, :].to_broadcast((128, LNH, 64))
                sin_b = sin_sb[:, tt:tt + 1, :].to_broadcast((128, LNH, 64))
                m1 = p1t.tile([128, LNH, 64], f32, tag="m1")
                m2 = p1t.tile([128, LNH, 64], f32, tag="m2")
                qkf = p1t.tile([128, LNH, 128], bf16, tag="qkf")
                nc.vector.tensor_tensor(m1[:], x1, cos_b, OP.mult)
                nc.vector.tensor_tensor(m2[:], x2, sin_b, OP.mult)
                nc.vector.tensor_tensor(qkf[:, :, 0:128:2], m1[:], m2[:], OP.subtract)
                nc.vector.tensor_tensor(m1[:], x2, cos_b, OP.mult)
                nc.vector.tensor_tensor(m2[:], x1, sin_b, OP.mult)
                nc.vector.tensor_tensor(qkf[:, :, 1:128:2], m1[:], m2[:], OP.add)

                # transpose each head tile [t,d] -> [d,t]
                for h5 in range(LNH):
                    pst = sps.tile([128, 128], bf16, tag="ss")
                    nc.tensor.transpose(pst[:], qkf[:, h5, :], ident_sb[:])
                    if h5 < QH:
                        nc.vector.tensor_copy(QT[:, h5, tt, :], pst[:])
                    else:
                        nc.vector.tensor_copy(KT[:, tt, :], pst[:])

                if tt % 4 == 3:
                    attn_block(tt // 4)

        # w_o loaded here so its DMA doesn't delay the P1 weight loads
        wo_r = wo.ap().rearrange("(ko p) n -> p ko n", p=128)
        wo_sb = const.tile([128, KO, 512], bf16)
        for c in range(4):
            nc.sync.dma_start(wo_sb[:, 8 * c:8 * (c + 1), :],
                              wo_r[:, 8 * c:8 * (c + 1), :])

        # ---- P5: o_proj ----
        # ko-outer so each stationary w_o tile serves all 4 token blocks
        # (128 LDWEIGHTS instead of 512); gathered attnT fully SBUF-resident.
        with tc.tile_pool(name="agp", bufs=4) as agp, \
             tc.tile_pool(name="osb", bufs=2) as osb, \
             tc.tile_pool(name="ops", bufs=2, space="PSUM") as ops:
            rts = []
            for tq in range(QC):
                rt = agp.tile([128, KO, 512], bf16, tag="rt", name=f"rt{tq}")
                agr = ag_out[tq].rearrange("(ko p) n -> p ko n", p=128)
                for c in range(8):
                    nc.sync.dma_start(rt[:, 4 * c:4 * (c + 1), :],
                                      agr[:, 4 * c:4 * (c + 1), :])
                rts.append(rt)
            for hc in range(4):
                po = [ops.tile([128, 512], f32, tag=f"po{tq}", name=f"po{tq}")
                      for tq in range(QC)]
                for ko in range(KO):
                    for tq in range(QC):
                        nc.tensor.matmul(po[tq][:],
                                         wo_sb[:, ko, hc * 128:(hc + 1) * 128],
                                         rts[tq][:, ko, :],
                                         start=(ko == 0), stop=(ko == KO - 1))
                for tq in range(QC):
                    ot = osb.tile([128, 512], f32, tag="ot")
                    nc.scalar.copy(ot[:], po[tq][:])
                    nc.sync.dma_start(
                        outT.ap()[hc * 128:(hc + 1) * 128, tq * 512:(tq + 1) * 512],
                        ot[:])

    nc.compile()
    return nc


def _prep_inputs(positions, hidden_states, w_qkv, w_o, q_norm_w, k_norm_w):
    hidden_states = np.asarray(hidden_states, dtype=np.float32)
    w_qkv = np.asarray(w_qkv, dtype=np.float32)
    w_o = np.asarray(w_o, dtype=np.float32)
    q_norm_w = np.asarray(q_norm_w, dtype=np.float32)
    k_norm_w = np.asarray(k_norm_w, dtype=np.float32)
    pos = np.asarray(positions).astype(np.float32)

    # hiddenT tiled for 8KB-contiguous per-partition DMA: [tt, p(H%128), ko, tl]
    hT = np.ascontiguousarray(
        hidden_states.reshape(TT, 128, KO, 128).transpose(0, 3, 2, 1)
    ).astype(BF16)

    inv_freq = THETA ** (-np.arange(64, dtype=np.float32) / 64.0)
    freqs = pos[:, None] * inv_freq[None, :]
    cos = np.cos(freqs).astype(np.float32).reshape(TT, 128, 64).transpose(1, 0, 2)
    sin = np.sin(freqs).astype(np.float32).reshape(TT, 128, 64).transpose(1, 0, 2)
    cos = np.ascontiguousarray(cos)
    sin = np.ascontiguousarray(sin)

    triu = np.triu(np.ones((128, 128), dtype=np.float32)).astype(BF16)
    identm = np.eye(128, dtype=np.float32).astype(BF16)
    onesm = np.ones((128, 128), dtype=np.float32).astype(BF16)

    in_maps = []
    for c in range(N_CORES):
        qcols = w_qkv[:, 4 * c * HD:(4 * c + 4) * HD]
        kcols = w_qkv[:, NH * HD + c * HD: NH * HD + (c + 1) * HD]
        vcols = w_qkv[:, (NH + NKV) * HD + c * HD: (NH + NKV) * HD + (c + 1) * HD]
        wqkv_sh = np.concatenate([qcols, kcols, vcols], axis=1).astype(BF16)
        wo_sh = np.ascontiguousarray(w_o[:, 512 * c:512 * (c + 1)]).astype(BF16)
        ln5 = np.concatenate([q_norm_w[4 * c:4 * c + 4], k_norm_w[c:c + 1]], axis=0)
        lnw_rep = np.ascontiguousarray(
            np.broadcast_to(ln5[None, :, :], (128, LNH, 128))).astype(np.float32)
        in_maps.append({
            "hT": hT,
            "wqkv": wqkv_sh,
            "wo": wo_sh,
            "cosd": cos,
            "sind": sin,
            "lnw": lnw_rep,
            "triu": triu,
            "ident": identm,
            "onesd": onesm,
        })
    return in_maps


def kernel(positions, hidden_states, w_qkv, w_o, q_norm_w, k_norm_w):
    from concourse.bass_utils import run_bass_kernel_spmd

    if "nc" not in _CACHE:
        _CACHE["nc"] = _build()
    nc = _CACHE["nc"]

    in_maps = _prep_inputs(positions, hidden_states, w_qkv, w_o,
                           q_norm_w, k_norm_w)
    res = run_bass_kernel_spmd(nc, in_maps, core_ids=list(range(N_CORES)))
    out = np.empty((T, H), dtype=np.float32)
    for c in range(N_CORES):
        out[:, 512 * c:512 * (c + 1)] = res.results[c]["outT"].T
    return out


# revision 26
# speedup vs baseline: 1.1036x; 1.1036x over previous
"""Trainium2 Bass kernel for CohereAttention (T=2048, H=4096, NH=32, NKV=8, HD=128).

Sharding: tensor-parallel across heads on 8 cores (SGLang-style).
  - core c owns q-heads [4c, 4c+4) and kv-head c (GQA rep=4 maps exactly).
  - w_qkv column-sharded per core -> [4096, 768] (4q|1k|1v head blocks).
  - attention output (bf16, d-major [512, 2048]) AllGather'd across cores.
  - w_o column-sharded -> each core computes a [2048, 512] column shard of the
    output (stored transposed [512, 2048]); host concatenates.

Device pipeline per core:
  P1  qkv = hiddenT.T @ w_qkv_shard (bf16 matmul, fp32 psum), per-head
      layernorm + interleaved (GPT-J) RoPE fused on DVE/ACT, V cast to bf16.
  P2  PE transposes q/k head tiles [t,d] -> [d,t] for the scores matmuls.
  P3  per head: scoresT[k,q] = K_d^T Q_d; probsT = exp(scale*s) (causal-masked);
      attnT[d,q] += V_k^T probsT; sums[q] += ones^T probsT (PSUM accumulate);
      attnT_norm = attnT * recip(sums) -> bf16 -> DRAM AllGather input.
  P4  AllGather (bf16, 2.1MB/core) -> [4096, 2048].
  P5  o_projT[hcol, t] += w_o[:, hcol]^T attnT_full, fp32 out.
"""

import numpy as np
import ml_dtypes

T = 2048
H = 4096
NH = 32
NKV = 8
HD = 128
N_CORES = 8
QH = NH // N_CORES          # q heads per core = 4
LNH = QH + 1                # layernormed heads per core (4 q + 1 k)
EPS = 1e-5
THETA = 10000.0
SCALE = HD ** -0.5
TT = T // 128               # 16 token tiles
KO = H // 128               # 32 contraction chunks
QC = T // 512               # 4 query chunks of 512
BF16 = ml_dtypes.bfloat16

_CACHE = {}


def _build():
    import concourse.bass as bass
    import concourse.mybir as mybir
    import concourse.tile as tile
    from concourse import bacc
    from contextlib import ExitStack

    dt = mybir.dt
    f32 = dt.float32
    bf16 = dt.bfloat16
    AX = mybir.AxisListType
    OP = mybir.AluOpType
    ACT = mybir.ActivationFunctionType

    nc = bacc.Bacc("TRN2", target_bir_lowering=False, debug=False,
                   num_devices=N_CORES)

    # ---- I/O ----
    hT = nc.dram_tensor("hT", [TT, 128, KO, 128], bf16, kind="ExternalInput")
    wqkv = nc.dram_tensor("wqkv", [H, 768], bf16, kind="ExternalInput")
    wo = nc.dram_tensor("wo", [H, 512], bf16, kind="ExternalInput")
    cosd = nc.dram_tensor("cosd", [128, TT, 64], f32, kind="ExternalInput")
    sind = nc.dram_tensor("sind", [128, TT, 64], f32, kind="ExternalInput")
    lnw = nc.dram_tensor("lnw", [128, LNH, 128], f32, kind="ExternalInput")
    triu = nc.dram_tensor("triu", [128, 128], bf16, kind="ExternalInput")
    ident = nc.dram_tensor("ident", [128, 128], bf16, kind="ExternalInput")
    onesd = nc.dram_tensor("onesd", [128, 128], bf16, kind="ExternalInput")
    outT = nc.dram_tensor("outT", [512, T], f32, kind="ExternalOutput")

    with tile.TileContext(nc) as tc, ExitStack() as ctx:
        const = ctx.enter_context(tc.tile_pool(name="const", bufs=1))
        dram = ctx.enter_context(tc.tile_pool(name="dram", bufs=1, space="DRAM"))

        cos_sb = const.tile([128, TT, 64], f32)
        nc.sync.dma_start(cos_sb[:], cosd.ap())
        sin_sb = const.tile([128, TT, 64], f32)
        nc.sync.dma_start(sin_sb[:], sind.ap())
        lnw_sb = const.tile([128, LNH, 128], f32)
        nc.sync.dma_start(lnw_sb[:], lnw.ap())
        triu_sb = const.tile([128, 128], bf16)
        nc.sync.dma_start(triu_sb[:], triu.ap())
        ident_sb = const.tile([128, 128], bf16)
        nc.sync.dma_start(ident_sb[:], ident.ap())
        ones_sb = const.tile([128, 128], bf16)
        nc.sync.dma_start(ones_sb[:], onesd.ap())
        eps_sb = const.tile([128, 1], f32)
        nc.vector.memset(eps_sb[:], EPS)

        ag_in = [dram.tile([QH * 128, 512], bf16, name=f"agi{i}")
                 for i in range(QC)]
        ag_out = [dram.tile([NH * 128, 512], bf16, addr_space="Shared",
                            name=f"ago{i}")
                  for i in range(QC)]

        # ---- P1: qkv projection + LN + RoPE + transposes, with attention
        # blocks interleaved (block qc emitted once token tiles <= 4qc+3 are
        # done, so its AllGather fires early and hides under P1 compute) ----
        with tc.tile_pool(name="htp", bufs=3) as htp, \
             tc.tile_pool(name="qkps", bufs=2, space="PSUM") as qkps, \
             tc.tile_pool(name="p1t", bufs=2) as p1t, \
             tc.tile_pool(name="sps", bufs=2, space="PSUM") as sps, \
             tc.tile_pool(name="pvsm", bufs=1, space="PSUM") as pvsm, \
             tc.tile_pool(name="probs", bufs=3) as probs, \
             tc.tile_pool(name="attn", bufs=2) as attn, \
             tc.tile_pool(name="acts", bufs=1) as acts:

            # persistent activations: d-major Q/K, t-major V (bf16)
            QT = acts.tile([128, QH, TT, 128], bf16)    # [d, h, tt, t]
            KT = acts.tile([128, TT, 128], bf16)        # [d, kt, t]
            Vt = acts.tile([128, TT, 128], bf16)        # [t, kt, d]

            def attn_block(qc):
                for h in range(QH):
                    pvs = pvsm.tile([128, 1024], f32, tag="pvsm")
                    pv = pvs[:, 0:512]
                    sm = pvs[:, 512:1024]
                    nkt = 4 * (qc + 1)
                    for kt in range(nkt):
                        # diagonal band: only q-subtiles j >= m are visible
                        m = max(kt - 4 * qc, 0)
                        lo = m * 128
                        ss = sps.tile([128, 512], f32, tag="ss")
                        nc.tensor.matmul(ss[:, lo:512], KT[:, kt, :],
                                         QT[:, h, 4 * qc + m:4 * qc + 4, :],
                                         start=True, stop=True)
                        pT = probs.tile([128, 4, 128], bf16, tag="pT")
                        pTf = pT.rearrange("p a b -> p (a b)")
                        nc.scalar.activation(pTf[:, lo:512], ss[:, lo:512],
                                             ACT.Exp, scale=SCALE)
                        if kt >= 4 * qc:
                            nc.vector.tensor_tensor(pT[:, m, :], pT[:, m, :],
                                                    triu_sb[:], OP.mult)
                        nc.tensor.matmul(pv[:, lo:512], Vt[:, kt, :],
                                         pTf[:, lo:512],
                                         start=(kt == 0), stop=(kt == nkt - 1))
                        nc.tensor.matmul(sm[:, lo:512], ones_sb[:],
                                         pTf[:, lo:512],
                                         start=(kt == 0), stop=(kt == nkt - 1))
                    recip = attn.tile([128, 512], f32, tag="recip")
                    nc.vector.reciprocal(recip[:], sm[:])
                    at = attn.tile([128, 512], bf16, tag="at")
                    nc.vector.tensor_tensor(at[:], pv[:], recip[:], OP.mult)
                    nc.sync.dma_start(ag_in[qc][h * 128:(h + 1) * 128, :], at[:])

                # AllGather this query block's attention output across cores
                nc.gpsimd.collective_compute(
                    "AllGather", mybir.AluOpType.bypass,
                    replica_groups=[list(range(N_CORES))],
                    ins=[ag_in[qc].opt()], outs=[ag_out[qc].opt()])

            wqkv_r = wqkv.ap().rearrange("(ko p) n -> p ko n", p=128)
            wqkv_sb = htp.tile([128, KO, 768], bf16, tag="wqkv", bufs=1)
            for c in range(8):
                nc.sync.dma_start(wqkv_sb[:, 4 * c:4 * (c + 1), :],
                                  wqkv_r[:, 4 * c:4 * (c + 1), :])
            for tt in range(TT):
                ht_t = htp.tile([128, KO, 128], bf16, tag="ht")
                nc.sync.dma_start(ht_t[:], hT.ap()[tt])
                ps = qkps.tile([128, 768], f32, tag="qk")
                for ko in range(KO):
                    nc.tensor.matmul(ps[:, 0:512], ht_t[:, ko, :],
                                     wqkv_sb[:, ko, 0:512],
                                     start=(ko == 0), stop=(ko == KO - 1))
                    nc.tensor.matmul(ps[:, 512:768], ht_t[:, ko, :],
                                     wqkv_sb[:, ko, 512:768],
                                     start=(ko == 0), stop=(ko == KO - 1))
                qkv_t = p1t.tile([128, 768], f32, tag="qkv")
                nc.scalar.copy(qkv_t[:], ps[:])

                # V: plain bf16 cast into persistent tile
                nc.vector.tensor_copy(Vt[:, tt, :], qkv_t[:, 640:768])

                # layernorm over the 5 q/k heads
                x5 = qkv_t[:, 0:640].rearrange("p (h d) -> p h d", d=128)
                mean = p1t.tile([128, LNH], f32, tag="mean")
                nc.vector.tensor_reduce(mean[:], x5, AX.X, OP.add)
                nc.scalar.mul(mean[:], mean[:], 1.0 / HD)
                xc = p1t.tile([128, LNH, 128], f32, tag="xc")
                nc.vector.tensor_tensor(
                    xc[:], x5, mean[:, :, None].to_broadcast((128, LNH, 128)),
                    OP.subtract)
                sq = p1t.tile([128, LNH, 128], f32, tag="sq")
                nc.vector.tensor_tensor(sq[:], xc[:], xc[:], OP.mult)
                var = p1t.tile([128, LNH], f32, tag="var")
                nc.vector.tensor_reduce(var[:], sq[:], AX.X, OP.add)
                std = p1t.tile([128, LNH], f32, tag="std")
                nc.scalar.activation(std[:], var[:], ACT.Sqrt,
                                     bias=eps_sb[:], scale=1.0 / HD)
                rstd = p1t.tile([128, LNH], f32, tag="rstd")
                nc.vector.reciprocal(rstd[:], std[:])
                nc.vector.tensor_tensor(
                    xc[:], xc[:], rstd[:, :, None].to_broadcast((128, LNH, 128)),
                    OP.mult)
                nc.vector.tensor_tensor(xc[:], xc[:], lnw_sb[:], OP.mult)

                # interleaved RoPE: out[2i] = x1*cos - x2*sin; out[2i+1] = x2*cos + x1*sin
                x1 = xc[:, :, 0:128:2]
                x2 = xc[:, :, 1:128:2]
                cos_b = cos_sb[:, tt:tt + 1, :].to_broadcast((128, LNH, 64))
                sin_b = sin_sb[:, tt:tt + 1, :].to_broadcast((128, LNH, 64))
                m1 = p1t.tile([128, LNH, 64], f32, tag="m1")
                m2 = p1t.tile([128, LNH, 64], f32, tag="m2")
                qkf = p1t.tile([128, LNH, 128], bf16, tag="qkf")
                nc.vector.tensor_tensor(m1[:], x1, cos_b, OP.mult)
                nc.vector.tensor_tensor(m2[:], x2, sin_b, OP.mult)
                nc.vector.tensor_tensor(qkf[:, :, 0:128:2], m1[:], m2[:], OP.subtract)
                nc.vector.tensor_tensor(m1[:], x2, cos_b, OP.mult)
                nc.vector.tensor_tensor(m2[:], x1, sin_b, OP.mult)
                nc.vector.tensor_tensor(qkf[:, :, 1:128:2], m1[:], m2[:], OP.add)

                # transpose each head tile [t,d] -> [d,t]
                for h5 in range(LNH):
                    pst = sps.tile([128, 128], bf16, tag="ss")
                    nc.tensor.transpose(pst[:], qkf[:, h5, :], ident_sb[:])
                    if h5 < QH:
                        nc.vector.tensor_copy(QT[:, h5, tt, :], pst[:])
                    else:
                        nc.vector.tensor_copy(KT[:, tt, :], pst[:])

                if tt % 4 == 3:
                    attn_block(tt // 4)

        # w_o loaded here so its DMA doesn't delay the P1 weight loads
        wo_r = wo.ap().rearrange("(ko p) n -> p ko n", p=128)
        wo_sb = const.tile([128, KO, 512], bf16)
        for c in range(4):
            nc.sync.dma_start(wo_sb[:, 8 * c:8 * (c + 1), :],
                              wo_r[:, 8 * c:8 * (c + 1), :])

        # ---- P5: o_proj ----
        with tc.tile_pool(name="agp", bufs=2) as agp, \
             tc.tile_pool(name="osb", bufs=2) as osb, \
             tc.tile_pool(name="ops", bufs=2, space="PSUM") as ops:
            for tq in range(QC):
                rt = agp.tile([128, KO, 512], bf16, tag="rt")
                agr = ag_out[tq].rearrange("(ko p) n -> p ko n", p=128)
                for c in range(8):
                    nc.sync.dma_start(rt[:, 4 * c:4 * (c + 1), :],
                                      agr[:, 4 * c:4 * (c + 1), :])
                for hc in range(4):
                    po = ops.tile([128, 512], f32, tag="po")
                    for ko in range(KO):
                        nc.tensor.matmul(po[:],
                                         wo_sb[:, ko, hc * 128:(hc + 1) * 128],
                                         rt[:, ko, :],
                                         start=(ko == 0), stop=(ko == KO - 1))
                    ot = osb.tile([128, 512], f32, tag="ot")
                    nc.scalar.copy(ot[:], po[:])
                    nc.sync.dma_start(
                        outT.ap()[hc * 128:(hc + 1) * 128, tq * 512:(tq + 1) * 512],
                        ot[:])

    nc.compile()
    return nc


def _prep_inputs(positions, hidden_states, w_qkv, w_o, q_norm_w, k_norm_w):
    hidden_states = np.asarray(hidden_states, dtype=np.float32)
    w_qkv = np.asarray(w_qkv, dtype=np.float32)
    w_o = np.asarray(w_o, dtype=np.float32)
    q_norm_w = np.asarray(q_norm_w, dtype=np.float32)
    k_norm_w = np.asarray(k_norm_w, dtype=np.float32)
    pos = np.asarray(positions).astype(np.float32)

    # hiddenT tiled for 8KB-contiguous per-partition DMA: [tt, p(H%128), ko, tl]
    hT = np.ascontiguousarray(
        hidden_states.reshape(TT, 128, KO, 128).transpose(0, 3, 2, 1)
    ).astype(BF16)

    inv_freq = THETA ** (-np.arange(64, dtype=np.float32) / 64.0)
    freqs = pos[:, None] * inv_freq[None, :]
    cos = np.cos(freqs).astype(np.float32).reshape(TT, 128, 64).transpose(1, 0, 2)
    sin = np.sin(freqs).astype(np.float32).reshape(TT, 128, 64).transpose(1, 0, 2)
    cos = np.ascontiguousarray(cos)
    sin = np.ascontiguousarray(sin)

    triu = np.triu(np.ones((128, 128), dtype=np.float32)).astype(BF16)
    identm = np.eye(128, dtype=np.float32).astype(BF16)
    onesm = np.ones((128, 128), dtype=np.float32).astype(BF16)

    in_maps = []
    for c in range(N_CORES):
        qcols = w_qkv[:, 4 * c * HD:(4 * c + 4) * HD]
        kcols = w_qkv[:, NH * HD + c * HD: NH * HD + (c + 1) * HD]
        vcols = w_qkv[:, (NH + NKV) * HD + c * HD: (NH + NKV) * HD + (c + 1) * HD]
        wqkv_sh = np.concatenate([qcols, kcols, vcols], axis=1).astype(BF16)
        wo_sh = np.ascontiguousarray(w_o[:, 512 * c:512 * (c + 1)]).astype(BF16)
        ln5 = np.concatenate([q_norm_w[4 * c:4 * c + 4], k_norm_w[c:c + 1]], axis=0)
        lnw_rep = np.ascontiguousarray(
            np.broadcast_to(ln5[None, :, :], (128, LNH, 128))).astype(np.float32)
        in_maps.append({
            "hT": hT,
            "wqkv": wqkv_sh,
            "wo": wo_sh,
            "cosd": cos,
            "sind": sin,
            "lnw": lnw_rep,
            "triu": triu,
            "ident": identm,
            "onesd": onesm,
        })
    return in_maps


def kernel(positions, hidden_states, w_qkv, w_o, q_norm_w, k_norm_w):
    from concourse.bass_utils import run_bass_kernel_spmd

    if "nc" not in _CACHE:
        _CACHE["nc"] = _build()
    nc = _CACHE["nc"]

    in_maps = _prep_inputs(positions, hidden_states, w_qkv, w_o,
                           q_norm_w, k_norm_w)
    res = run_bass_kernel_spmd(nc, in_maps, core_ids=list(range(N_CORES)))
    out = np.empty((T, H), dtype=np.float32)
    for c in range(N_CORES):
        out[:, 512 * c:512 * (c + 1)] = res.results[c]["outT"].T
    return out
